# revision 8
# baseline (speedup 1.0000x reference)
"""AttentionSequencePoolingLayer (DIN attention) on 8 trn2 NeuronCores.

Data-parallel over batch: B=2048 -> 256 per core.  bf16 compute, f32 psum.

Math per (b,t):  att = concat([q,k,q-k,q*k]) @ W1 + b1
  Using row-blocks W1 = [W1a;W1b;W1c;W1d]:
    att = k @ (Bw + diag(q_b) C) + (q_b@A + b1)
  Folded per-batch weight Wf_b [64,80], bias row U_b appended -> lhsT [65,80]
  against keys augmented with a ones-row [65,L].  So no per-batch bias op.
  h1 = sigmoid(att); h2 = sigmoid(h1@W2+b2); s = h2@W3 + b3; out = s^T K.

Masking: host zeroes keys at t >= keys_length in BOTH layouts.  Scores at
masked positions then multiply zero key vectors in the pooling matmul, so
their contribution is exactly 0 == reference's where(mask, scores, 0).

Length-sorted batching: host argsorts batches by keys_length; slot i on
core c holds sorted batch order[8i+c], processed with a static per-slot
token count L[i] = max over the 8 cores (so the single SPMD program fits
every core; keys are zero-padded up to L[i], which the masking trick makes
exact).  Tokens per slot are packed back-to-back in the free dim, cutting
PE/Act/DMA work roughly in half vs. fixed T=200.

Per-slot engine work (L = padded token count):
  PE:  L1 [65,80]x[65,L] + L2 [80,40]x[80,L] + L3 [40,<=128]x[40,1] per
       128-token chunk + pooling [chunk,1]x[chunk,64] per chunk
  Act: sigmoid over groups of slots packed into one psum bank (<=512 cols)
  DVE: scores + b3 -> bf16 once per 32-slot tile; output copies
Pooling for tile i is emitted during tile i+1's MLP to keep PE dense.
"""
import numpy as np
import ml_dtypes

import concourse.bacc as bacc
import concourse.bass as bass
import concourse.mybir as mybir
import concourse.tile as tile
from concourse.bass_utils import run_bass_kernel_spmd

B, T, E = 2048, 200, 64
H1, H2 = 80, 40
NCORES = 8
BL = B // NCORES          # 256 slots per core
BT = 32                   # slots per tile
NT = BL // BT             # 8 tiles
KA = E + 1                # augmented contraction (keys + ones row)
CH = 128                  # pooling chunk (psum partition limit)
BANK = 512                # psum bank capacity in f32 cols

BF16 = ml_dtypes.bfloat16

_cache = {}


def _schedule(L):
    """Static per-core schedule from padded slot lengths L[256]."""
    L = list(L)
    nch = [1 if l <= CH else 2 for l in L]
    chunk0 = [min(l, CH) for l in L]
    chunk1 = [l - CH if l > CH else 0 for l in L]
    # chunk-block column index in the packed kn array, per slot
    cb = np.concatenate([[0], np.cumsum(nch)]).astype(int)
    # per-tile in-tile col offsets and L1 groups (consecutive slots, <=512)
    tiles = []
    for t in range(NT):
        s0 = t * BT
        Ls = L[s0 : s0 + BT]
        off = np.concatenate([[0], np.cumsum(Ls)]).astype(int)
        groups = []
        cur, acc = [], 0
        for j in range(BT):
            if cur and acc + Ls[j] > BANK:
                groups.append(cur)
                cur, acc = [], 0
            cur.append(j)
            acc += Ls[j]
        if cur:
            groups.append(cur)
        tiles.append({
            "s0": s0, "Ls": Ls, "off": off, "W": int(off[-1]),
            "groups": groups,
            "c0": int(cb[s0]), "nc": int(cb[s0 + BT] - cb[s0]),
        })
    return {
        "L": L, "nch": nch, "chunk0": chunk0, "chunk1": chunk1,
        "cb": cb, "tiles": tiles,
        "TOT": int(sum(L)), "NC": int(cb[-1]),
    }


def _build(b3f: float, sched):
    nc = bacc.Bacc(None, target_bir_lowering=False)
    f32 = mybir.dt.float32
    bf16 = mybir.dt.bfloat16

    TOT, NC = sched["TOT"], sched["NC"]
    kfa_d = nc.dram_tensor("kfa", [KA, TOT], bf16, kind="ExternalInput")
    kn_d = nc.dram_tensor("kn", [CH, NC, E], bf16, kind="ExternalInput")
    wfa_d = nc.dram_tensor("wfa", [KA, BL, H1], bf16, kind="ExternalInput")
    w2_d = nc.dram_tensor("w2", [H1, H2], bf16, kind="ExternalInput")
    w3d_d = nc.dram_tensor("w3d", [104, 1], bf16, kind="ExternalInput")
    b2d_d = nc.dram_tensor("b2d", [104, 1], f32, kind="ExternalInput")
    out_d = nc.dram_tensor("out", [BL * E], bf16, kind="ExternalOutput")

    SIG = mybir.ActivationFunctionType.Sigmoid
    ADD = mybir.AluOpType.add

    with tile.TileContext(nc) as tc:
        with (
            tc.tile_pool(name="big", bufs=2) as big,
            tc.tile_pool(name="knp", bufs=3) as knp,
            tc.tile_pool(name="const", bufs=1) as const,
            tc.tile_pool(name="h1p", bufs=3) as h1p,
            tc.tile_pool(name="h2p", bufs=2) as h2p,
            tc.tile_pool(name="stm", bufs=2) as stm,
            tc.tile_pool(name="p1", bufs=2, space=bass.MemorySpace.PSUM) as p1p,
            tc.tile_pool(name="p2", bufs=2, space=bass.MemorySpace.PSUM) as p2p,
            tc.tile_pool(name="pS", bufs=2, space=bass.MemorySpace.PSUM) as pSp,
            tc.tile_pool(name="po", bufs=2, space=bass.MemorySpace.PSUM) as pop,
        ):
            w2_s = const.tile([H1, H2], bf16)
            w3_s = const.tile([104, 1], bf16)
            b2_s = const.tile([104, 1], f32)
            nc.sync.dma_start(w2_s[:], w2_d[:])
            nc.sync.dma_start(w3_s[:], w3d_d[:])
            nc.sync.dma_start(b2_s[:], b2d_d[:])

            def emit_pooling(STm, knt, tl):
                s0 = tl["s0"]
                for j in range(BT):
                    i = s0 + j
                    if j % 8 == 0:
                        pout = pop.tile([1, 8, E], f32, tag="pout")
                    nchunks = sched["nch"][i]
                    for ci in range(nchunks):
                        cl = sched["chunk0"][i] if ci == 0 else sched["chunk1"][i]
                        kcol = sched["cb"][i] - tl["c0"] + ci
                        nc.tensor.matmul(
                            pout[0:1, j % 8, :],
                            STm[0:cl, ci, j : j + 1],
                            knt[0:cl, kcol, :],
                            start=(ci == 0),
                            stop=(ci == nchunks - 1),
                        )
                    if j % 8 == 7:
                        orow = stm.tile([1, 8, E], bf16, tag="orow")
                        nc.vector.tensor_copy(orow[:], pout[:])
                        g0 = i - 7
                        nc.sync.dma_start(
                            out_d[g0 * E : (g0 + 8) * E], orow[:]
                        )

            prev = None
            ktot = 0
            for t in range(NT):
                tl = sched["tiles"][t]
                s0, Ls, off, W = tl["s0"], tl["Ls"], tl["off"], tl["W"]
                kf = big.tile([KA, W], bf16, tag=f"kf{t % 2}")
                knt = knp.tile([CH, tl["nc"], E], bf16, tag=f"kn{t % 3}")
                wf = big.tile([KA, BT, H1], bf16, tag="wf")
                nc.sync.dma_start(kf[:], kfa_d[:, ktot : ktot + W])
                nc.sync.dma_start(
                    knt[:], kn_d[:, tl["c0"] : tl["c0"] + tl["nc"], :]
                )
                nc.sync.dma_start(wf[:], wfa_d[:, s0 : s0 + BT, :])
                ktot += W

                Sps = pSp.tile([CH, 2, BT], f32, tag="Sps")
                nc.vector.memset(Sps[:], 0.0)

                groups = tl["groups"]
                for gp in range(0, len(groups), 2):
                    pair = groups[gp : gp + 2]
                    h1g = []
                    gws = []
                    for g in pair:
                        gw = int(sum(Ls[j] for j in g))
                        go0 = int(off[g[0]])
                        att = p1p.tile([H1, gw], f32, tag="att")
                        for j in g:
                            o = int(off[j]) - go0
                            nc.tensor.matmul(
                                att[:, o : o + Ls[j]],
                                wf[:, j, :],
                                kf[:, int(off[j]) : int(off[j]) + Ls[j]],
                                start=True, stop=True,
                            )
                        h1 = h1p.tile([H1, gw], bf16, tag="h1")
                        nc.scalar.activation(h1[:], att[:], SIG)
                        h1g.append(h1)
                        gws.append(gw)
                    gwm = max(gws)
                    g2 = p2p.tile([104, gwm], f32, tag="g2")
                    if len(gws) == 2 and gws[0] != gws[1]:
                        a = int(np.argmin(gws))
                        p0 = 64 * a
                        nc.vector.memset(g2[p0 : p0 + H2, min(gws) : gwm], 0.0)
                    for a, g in enumerate(pair):
                        p0 = 64 * a
                        go0 = int(off[g[0]])
                        for j in g:
                            o = int(off[j]) - go0
                            nc.tensor.matmul(
                                g2[p0 : p0 + H2, o : o + Ls[j]],
                                w2_s[:],
                                h1g[a][:, o : o + Ls[j]],
                                start=True, stop=True,
                            )
                    h2 = h2p.tile([104, gwm], bf16, tag="h2")
                    nc.scalar.activation(h2[:], g2[:], SIG, bias=b2_s[:, 0:1])
                    for a, g in enumerate(pair):
                        p0 = 64 * a
                        go0 = int(off[g[0]])
                        for j in g:
                            i = s0 + j
                            o = int(off[j]) - go0
                            for ci in range(sched["nch"][i]):
                                c0 = ci * CH
                                cl = (sched["chunk0"][i] if ci == 0
                                      else sched["chunk1"][i])
                                nc.tensor.matmul(
                                    Sps[0:cl, ci, j : j + 1],
                                    h2[p0 : p0 + H2, o + c0 : o + c0 + cl],
                                    w3_s[p0 : p0 + H2, :],
                                    start=True, stop=True,
                                )
                STm = stm.tile([CH, 2, BT], bf16, tag="STm")
                nc.vector.tensor_scalar(STm[:], Sps[:], b3f, None, ADD)
                if prev is not None:
                    emit_pooling(*prev)
                prev = (STm, knt, tl)
            emit_pooling(*prev)

    nc.compile()
    return nc


def kernel(query, keys, keys_length, W1, b1, W2, b2, W3, b3):
    query = np.asarray(query, np.float32)
    keys = np.asarray(keys, np.float32)
    keys_length = np.asarray(keys_length, np.int32)
    W1 = np.asarray(W1, np.float32); b1 = np.asarray(b1, np.float32)
    W2 = np.asarray(W2, np.float32); b2 = np.asarray(b2, np.float32)
    W3 = np.asarray(W3, np.float32); b3 = np.asarray(b3, np.float32)

    A = W1[0:E] + W1[2 * E : 3 * E]          # q coeff
    Bw = W1[E : 2 * E] - W1[2 * E : 3 * E]   # k coeff
    C = W1[3 * E : 4 * E]                    # q*k coeff

    q2 = query[:, 0, :]                      # [B, E]
    U = q2 @ A + b1                          # [B, H1]
    Wf = Bw[None, :, :] + q2[:, :, None] * C[None, :, :]
    wfa = np.concatenate([Wf, U[:, None, :]], 1).astype(BF16)  # [B, 65, 80]
    lens = keys_length[:, 0].astype(np.int64)
    mask = (np.arange(T)[None, :] < lens[:, None]).astype(np.float32)
    kz = (keys * mask[:, :, None]).astype(BF16)                # [B, T, E]

    # length-sorted slot assignment: slot i, core c -> batch order[8i+c]
    order = np.argsort(lens, kind="stable")
    omat = order.reshape(BL, NCORES)
    L = [int(l) for l in lens[order].reshape(BL, NCORES).max(1)]

    sched = _schedule(L)
    b3f = float(b3.reshape(-1)[0])
    key = ("v3", b3f, tuple(L))
    if key not in _cache:
        _cache[key] = _build(b3f, sched)
    nc = _cache[key]

    TOT, NC, cb = sched["TOT"], sched["NC"], sched["cb"]
    w2b = W2.astype(BF16)
    w3d = np.zeros((104, 1), BF16)
    w3d[0:H2, 0] = W3[:, 0].astype(BF16)
    w3d[64 : 64 + H2, 0] = W3[:, 0].astype(BF16)
    b2d = np.zeros((104, 1), np.float32)
    b2d[0:H2, 0] = b2
    b2d[64 : 64 + H2, 0] = b2

    in_maps = []
    for c in range(NCORES):
        bidx = omat[:, c]                    # batch per slot
        kzc = kz[bidx]                       # [BL, T, E] slot-ordered
        kfa = np.zeros((KA, TOT), BF16)
        kn = np.zeros((CH, NC, E), BF16)
        o = 0
        for i in range(BL):
            l = L[i]
            kfa[0:E, o : o + l] = kzc[i, 0:l, :].T
            kfa[E, o : o + l] = 1.0
            o += l
            c0 = cb[i]
            kn[0 : min(l, CH), c0, :] = kzc[i, 0 : min(l, CH), :]
            if l > CH:
                kn[0 : l - CH, c0 + 1, :] = kzc[i, CH:l, :]
        wfa_t = np.ascontiguousarray(wfa[bidx].transpose(1, 0, 2))
        in_maps.append({
            "kfa": kfa, "kn": kn, "wfa": wfa_t,
            "w2": w2b, "w3d": w3d, "b2d": b2d,
        })

    res = run_bass_kernel_spmd(nc, in_maps, list(range(NCORES)))
    _cache["last_res"] = res
    outs = [np.asarray(r["out"]).astype(np.float32).reshape(BL, E)
            for r in res.results]
    full = np.zeros((B, E), np.float32)
    for c in range(NCORES):
        full[omat[:, c]] = outs[c]
    return full.reshape(B, 1, E)


# revision 22
# speedup vs baseline: 1.2402x; 1.2402x over previous
"""AttentionSequencePoolingLayer (DIN attention) on 8 trn2 NeuronCores.

Data-parallel over batch: B=2048 -> 256 per core.  bf16 compute, f32 psum.

Math per (b,t):  att = concat([q,k,q-k,q*k]) @ W1 + b1
  Using row-blocks W1 = [W1a;W1b;W1c;W1d]:
    att = k @ (Bw + diag(q_b) C) + (q_b@A + b1)
  Folded per-batch weight Wf_b [64,80], bias row U_b appended -> lhsT [65,80]
  against keys augmented with a ones-row [65,L].  So no per-batch bias op.
  h1 = sigmoid(att); h2 = sigmoid(h1@W2+b2); s = h2@W3 + b3; out = s^T K.

Masking: host zeroes keys at t >= keys_length in BOTH layouts.  Scores at
masked positions then multiply zero key vectors in the pooling matmul, so
their contribution is exactly 0 == reference's where(mask, scores, 0).

Length-sorted batching: host argsorts batches by keys_length; slot i on
core c holds sorted batch order[8i+c], processed with a static per-slot
token count L[i] = max over the 8 cores (one SPMD program fits every core;
keys are zero-padded up to L[i], which the masking trick makes exact).

Slots are packed into groups of <=1024 token-columns (2 psum banks; a
slot's columns never straddle the mid-bank boundary -- padding gaps are
memset) so each sigmoid instruction covers a whole group: Act-engine
per-instruction access latency is the main overhead at this size.
L2 packs two groups at partition offsets 0/64 via PE column tiling.
Pooled outputs for 32 slots share one psum bank via column positions
{0,32,64,96}.  Scores+pooling are pipelined one group-pair behind the
MLP to keep PE/Act dense through the tail.
"""
import numpy as np
import ml_dtypes

import concourse.bacc as bacc
import concourse.bass as bass
import concourse.mybir as mybir
import concourse.tile as tile
from concourse.bass_utils import run_bass_kernel_spmd

B, T, E = 2048, 200, 64
H1, H2 = 80, 40
NCORES = 8
BL = B // NCORES          # 256 slots per core
BT = 32                   # slots per tile
NT = BL // BT             # 8 tiles
KA = E + 1                # augmented contraction (keys + ones row)
CH = 128                  # pooling chunk (psum partition limit)
BANK = 512                # psum bank capacity in f32 cols
GRP = 2 * BANK            # group capacity (2 banks)

BF16 = ml_dtypes.bfloat16

_cache = {}


def _schedule(L):
    """Static per-core schedule from padded slot lengths L[256].

    Slots are packed into groups of <=BANK cols; tiles are runs of up to
    TGRPS consecutive groups (and <=TSLOTS slots) sharing one set of DMAs.
    """
    L = list(L)
    nch = [1 if l <= CH else 2 for l in L]
    chunk0 = [min(l, CH) for l in L]
    chunk1 = [l - CH if l > CH else 0 for l in L]
    cb = np.concatenate([[0], np.cumsum(nch)]).astype(int)

    # global greedy groups (absolute slot ids)
    groups = []
    cur, acc = [], 0
    for i in range(BL):
        if cur and acc + L[i] > BANK:
            groups.append(cur)
            cur, acc = [], 0
        cur.append(i)
        acc += L[i]
    if cur:
        groups.append(cur)

    # tiles = runs of groups
    tiles = []
    g0 = 0
    while g0 < len(groups):
        g1 = g0
        slots = []
        while (g1 < len(groups) and g1 - g0 < TGRPS
               and len(slots) + len(groups[g1]) <= TSLOTS):
            slots.extend(groups[g1])
            g1 += 1
        if g1 == g0:        # single huge group
            slots = list(groups[g0])
            g1 = g0 + 1
        s0 = slots[0]
        Ls = [L[i] for i in slots]
        doff = np.concatenate([[0], np.cumsum(Ls)]).astype(int)
        gl = []
        for g in groups[g0:g1]:
            offs = []
            o = 0
            for i in g:
                offs.append(o)
                o += L[i]
            gl.append({"slots": [i - s0 for i in g], "offs": offs, "gw": o})
        tiles.append({
            "s0": s0, "ns": len(slots), "Ls": Ls, "doff": doff,
            "W": int(doff[-1]), "groups": gl,
            "c0": int(cb[s0]), "nc": int(cb[s0 + len(slots)] - cb[s0]),
        })
        g0 = g1
    return {
        "L": L, "nch": nch, "chunk0": chunk0, "chunk1": chunk1,
        "cb": cb, "tiles": tiles, "ntiles": len(tiles),
        "TOT": int(sum(L)), "NC": int(cb[-1]),
    }


def _build(b3f: float, sched):
    nc = bacc.Bacc(None, target_bir_lowering=False)
    f32 = mybir.dt.float32
    bf16 = mybir.dt.bfloat16

    TOT, NC = sched["TOT"], sched["NC"]
    kfa_d = nc.dram_tensor("kfa", [KA, TOT], bf16, kind="ExternalInput")
    kn_d = nc.dram_tensor("kn", [CH, NC, E], bf16, kind="ExternalInput")
    wfa_d = nc.dram_tensor("wfa", [KA, BL, H1], bf16, kind="ExternalInput")
    w2_d = nc.dram_tensor("w2", [H1, H2], bf16, kind="ExternalInput")
    w3d_d = nc.dram_tensor("w3d", [104, 1], bf16, kind="ExternalInput")
    b2d_d = nc.dram_tensor("b2d", [104, 1], f32, kind="ExternalInput")
    out_d = nc.dram_tensor("out", [BL * E], bf16, kind="ExternalOutput")

    SIG = mybir.ActivationFunctionType.Sigmoid
    ADD = mybir.AluOpType.add

    with tile.TileContext(nc) as tc:
        with (
            tc.tile_pool(name="big", bufs=3) as big,
            tc.tile_pool(name="knp", bufs=4) as knp,
            tc.tile_pool(name="const", bufs=1) as const,
            tc.tile_pool(name="h1p", bufs=3) as h1p,
            tc.tile_pool(name="h2p", bufs=2) as h2p,
            tc.tile_pool(name="stm", bufs=3) as stm,
            tc.tile_pool(name="orp", bufs=2) as orp,
            tc.tile_pool(name="p1", bufs=2, space=bass.MemorySpace.PSUM) as p1p,
            tc.tile_pool(name="p2", bufs=1, space=bass.MemorySpace.PSUM) as p2p,
            tc.tile_pool(name="pS", bufs=1, space=bass.MemorySpace.PSUM) as pSp,
            tc.tile_pool(name="po", bufs=1, space=bass.MemorySpace.PSUM) as pop,
        ):
            w2_s = const.tile([H1, H2], bf16)
            w3_s = const.tile([104, 1], bf16)
            b2_s = const.tile([104, 1], f32)

            # one pooling/score unit per group-pair, pipelined one unit
            # behind the MLP; one psum bank (partition positions 0/32) and
            # one output DMA per 16 slots
            pending = []          # (STm tile, knt tile, tl, jlist)
            state = {"pout": None, "orow": None}

            def pool_unit():
                STm, knt, tl, jlist = pending.pop(0)
                s0 = tl["s0"]
                for j in jlist:
                    i = s0 + j
                    if i % 16 == 0:
                        state["pout"] = pop.tile(
                            [33, 8, E], f32, tag="pout", name="pout"
                        )
                        state["orow"] = orp.tile(
                            [1, 16, E], bf16, tag="orow", name="orow"
                        )
                    pout, orow = state["pout"], state["orow"]
                    a, r = divmod(i % 16, 8)
                    nchunks = sched["nch"][i]
                    for ci in range(nchunks):
                        cl = (sched["chunk0"][i] if ci == 0
                              else sched["chunk1"][i])
                        kcol = sched["cb"][i] - tl["c0"] + ci
                        nc.tensor.matmul(
                            pout[32 * a : 32 * a + 1, r, :],
                            STm[0:cl, ci, j - jlist[0] : j - jlist[0] + 1],
                            knt[0:cl, kcol, :],
                            start=(ci == 0),
                            stop=(ci == nchunks - 1),
                        )
                    if i % 8 == 7:
                        nc.vector.tensor_copy(
                            orow[0:1, 8 * a : 8 * a + 8, :],
                            pout[32 * a : 32 * a + 1, :, :],
                        )
                    if i % 16 == 15:
                        g0 = i - 15
                        nc.sync.dma_start(
                            out_d[g0 * E : (g0 + 16) * E], orow[:]
                        )

            first = True
            ktot = 0
            for t in range(NT):
                tl = sched["tiles"][t]
                s0, Ls, doff, W = tl["s0"], tl["Ls"], tl["doff"], tl["W"]
                kf = big.tile([KA, W], bf16, tag=f"kf{t % 2}")
                knt = knp.tile([CH, tl["nc"], E], bf16, tag=f"kn{t % 3}")
                wf = big.tile([KA, BT, H1], bf16, tag="wf")
                nc.sync.dma_start(kf[:], kfa_d[:, ktot : ktot + W])
                nc.sync.dma_start(
                    knt[:], kn_d[:, tl["c0"] : tl["c0"] + tl["nc"], :]
                )
                nc.sync.dma_start(wf[:], wfa_d[:, s0 : s0 + BT, :])
                ktot += W
                if first:
                    # consts after the first tile's loads: L1 needs only kf+wf
                    nc.sync.dma_start(w2_s[:], w2_d[:])
                    nc.sync.dma_start(w3_s[:], w3d_d[:])
                    nc.sync.dma_start(b2_s[:], b2d_d[:])
                    first = False

                Sps = pSp.tile([CH, 2, BT], f32, tag="Sps")
                nc.vector.memset(Sps[:], 0.0)

                groups = tl["groups"]
                for gp in range(0, len(groups), 2):
                    pair = groups[gp : gp + 2]
                    h1g, jlist = [], []
                    for g in pair:
                        att = p1p.tile([H1, GRP], f32, tag="att")
                        for o, ln in g["gaps"]:
                            nc.vector.memset(att[:, o : o + ln], 0.0)
                        for j, o in zip(g["slots"], g["offs"]):
                            nc.tensor.matmul(
                                att[:, o : o + Ls[j]],
                                wf[:, j, :],
                                kf[:, int(doff[j]) : int(doff[j]) + Ls[j]],
                                start=True, stop=True,
                            )
                        h1 = h1p.tile([H1, GRP], bf16, tag="h1")
                        nc.scalar.activation(
                            h1[:, 0 : g["gw"]], att[:, 0 : g["gw"]], SIG
                        )
                        h1g.append(h1)
                        jlist.extend(g["slots"])
                    gws = [g["gw"] for g in pair]
                    gwm = max(gws)
                    g2 = p2p.tile([104, GRP], f32, tag="g2")
                    for a, g in enumerate(pair):
                        p0 = 64 * a
                        for o, ln in g["gaps"]:
                            nc.vector.memset(
                                g2[p0 : p0 + H2, o : o + ln], 0.0
                            )
                        if g["gw"] < gwm:
                            nc.vector.memset(
                                g2[p0 : p0 + H2, g["gw"] : gwm], 0.0
                            )
                        for j, o in zip(g["slots"], g["offs"]):
                            nc.tensor.matmul(
                                g2[p0 : p0 + H2, o : o + Ls[j]],
                                w2_s[:],
                                h1g[a][:, o : o + Ls[j]],
                                start=True, stop=True,
                            )
                    h2 = h2p.tile([104, GRP], bf16, tag="h2")
                    npart = 104 if len(pair) == 2 else H2
                    nc.scalar.activation(
                        h2[0:npart, 0:gwm], g2[0:npart, 0:gwm], SIG,
                        bias=b2_s[0:npart, 0:1],
                    )
                    for a, g in enumerate(pair):
                        p0 = 64 * a
                        for j, o in zip(g["slots"], g["offs"]):
                            i = s0 + j
                            for ci in range(sched["nch"][i]):
                                c0 = ci * CH
                                cl = (sched["chunk0"][i] if ci == 0
                                      else sched["chunk1"][i])
                                nc.tensor.matmul(
                                    Sps[0:cl, ci, j : j + 1],
                                    h2[p0 : p0 + H2, o + c0 : o + c0 + cl],
                                    w3_s[p0 : p0 + H2, :],
                                    start=True, stop=True,
                                )
                    j0, j1 = jlist[0], jlist[-1] + 1
                    STm = stm.tile([CH, 2, j1 - j0], bf16, tag="STm")
                    nc.vector.tensor_scalar(
                        STm[:], Sps[:, :, j0:j1], b3f, None, ADD
                    )
                    pending.append((STm, knt, tl, jlist))
                    if len(pending) > 1:
                        pool_unit()
            while pending:
                pool_unit()

    nc.compile()
    return nc


def kernel(query, keys, keys_length, W1, b1, W2, b2, W3, b3):
    query = np.asarray(query, np.float32)
    keys = np.asarray(keys, np.float32)
    keys_length = np.asarray(keys_length, np.int32)
    W1 = np.asarray(W1, np.float32); b1 = np.asarray(b1, np.float32)
    W2 = np.asarray(W2, np.float32); b2 = np.asarray(b2, np.float32)
    W3 = np.asarray(W3, np.float32); b3 = np.asarray(b3, np.float32)

    A = W1[0:E] + W1[2 * E : 3 * E]          # q coeff
    Bw = W1[E : 2 * E] - W1[2 * E : 3 * E]   # k coeff
    C = W1[3 * E : 4 * E]                    # q*k coeff

    q2 = query[:, 0, :]                      # [B, E]
    U = q2 @ A + b1                          # [B, H1]
    Wf = Bw[None, :, :] + q2[:, :, None] * C[None, :, :]
    wfa = np.concatenate([Wf, U[:, None, :]], 1).astype(BF16)  # [B, 65, 80]
    lens = keys_length[:, 0].astype(np.int64)
    mask = (np.arange(T)[None, :] < lens[:, None]).astype(np.float32)
    kz = (keys * mask[:, :, None]).astype(BF16)                # [B, T, E]

    # length-sorted slot assignment: slot i, core c -> batch order[8i+c]
    order = np.argsort(lens, kind="stable")
    omat = order.reshape(BL, NCORES)
    L = [int(l) for l in lens[order].reshape(BL, NCORES).max(1)]

    sched = _schedule(L)
    b3f = float(b3.reshape(-1)[0])
    key = ("v4", b3f, tuple(L))
    if key not in _cache:
        _cache[key] = _build(b3f, sched)
    nc = _cache[key]

    TOT, NC, cb = sched["TOT"], sched["NC"], sched["cb"]
    w2b = W2.astype(BF16)
    w3d = np.zeros((104, 1), BF16)
    w3d[0:H2, 0] = W3[:, 0].astype(BF16)
    w3d[64 : 64 + H2, 0] = W3[:, 0].astype(BF16)
    b2d = np.zeros((104, 1), np.float32)
    b2d[0:H2, 0] = b2
    b2d[64 : 64 + H2, 0] = b2

    in_maps = []
    for c in range(NCORES):
        bidx = omat[:, c]                    # batch per slot
        kzc = kz[bidx]                       # [BL, T, E] slot-ordered
        kfa = np.zeros((KA, TOT), BF16)
        kn = np.zeros((CH, NC, E), BF16)
        o = 0
        for i in range(BL):
            l = L[i]
            kfa[0:E, o : o + l] = kzc[i, 0:l, :].T
            kfa[E, o : o + l] = 1.0
            o += l
            c0 = cb[i]
            kn[0 : min(l, CH), c0, :] = kzc[i, 0 : min(l, CH), :]
            if l > CH:
                kn[0 : l - CH, c0 + 1, :] = kzc[i, CH:l, :]
        wfa_t = np.ascontiguousarray(wfa[bidx].transpose(1, 0, 2))
        in_maps.append({
            "kfa": kfa, "kn": kn, "wfa": wfa_t,
            "w2": w2b, "w3d": w3d, "b2d": b2d,
        })

    res = run_bass_kernel_spmd(nc, in_maps, list(range(NCORES)))
    _cache["last_res"] = res
    outs = [np.asarray(r["out"]).astype(np.float32).reshape(BL, E)
            for r in res.results]
    full = np.zeros((B, E), np.float32)
    for c in range(NCORES):
        full[omat[:, c]] = outs[c]
    return full.reshape(B, 1, E)
